# revision 72
# baseline (speedup 1.0000x reference)
"""Multi-head causal attention (N=4, T=2048, DM=1024, H=16, D=64) on 8 trn2 cores.

Sharding: core = (batch, head-half). Each core computes 8 of the 16 heads for
one batch over ALL 2048 queries: Q/K/V projections restricted to its 512
features, full causal attention for its heads, and a PARTIAL output
projection attn_g @ Wo[g-rows]. The two cores of a batch produce partial Y's
that the host sums (O-projection is linear over head groups) - no collectives
and no duplicated projection work.

On-chip: activations arrive host-transposed x^T [dm, tok] bf16 so Q/K project
directly to [feat, tok] (weights stationary); V projects naturally [tok, feat].
Scores are St[k, q] = Kt_h^T . Qt_h per key block, one fused exp (ACT) per
(head, q-half, kb) with the key-padding bias per partition; the causal
diagonal is a binary bf16 mask multiplied in on DVE. The A.V matmul is
FLIPPED: stationary = wt block [128k,128q], moving = V slice [128k,64] plus
a ones column producing Z - so each accumulation step charges only 65 output
columns. The 8 query-block accumulators share two PSUM banks; since
start=True clears the has_written bits of a WHOLE bank, only the round's
first matmul per bank starts (other regions' first writes rely on the
per-element overwrite-where-unset semantics). Per finished query block, a
reciprocal + tensor_scalar normalize (DVE) feeds a PE transpose back to
[feat, tok] for the output projection. Projections and O-proj chunks are
emitted as deadline/deficit-paced FILLER between attention steps (software-
pipelined scores->exp->A.V) so the in-order PE never drains while ACT runs
exp.
"""

import numpy as np
import ml_dtypes

import concourse.mybir as mybir
from concourse import bacc
from concourse.tile import TileContext
from concourse.bass_utils import run_bass_kernel_spmd

bf16 = mybir.dt.bfloat16
f32 = mybir.dt.float32

_NC_CACHE = {}
DEBUG_DUMP = False   # add QT/KT/V/ATN dram dumps for debugging

# T tokens, DM model dim, HG heads per core (group), D head dim
BUILD_KEY = (2048, 1024, 8, False)


def build_kernel(T, dm, hg, bias_mode):
    d = 64
    NB = T // 128          # token blocks (16)
    C = dm // 128          # contraction chunks (8)
    NP = hg // 2           # head pairs per core (4)
    FW = hg * d            # feature width per core (512)
    HB = NB // 2           # blocks per half (8)

    nc = bacc.Bacc("TRN2", target_bir_lowering=False, debug=False)

    xqT = nc.dram_tensor("xqT", [dm, T], bf16, kind="ExternalInput")
    xkT = nc.dram_tensor("xkT", [dm, T], bf16, kind="ExternalInput")
    xvT = nc.dram_tensor("xvT", [dm, T], bf16, kind="ExternalInput")
    Wq = nc.dram_tensor("Wq", [dm, FW], bf16, kind="ExternalInput")
    Wk = nc.dram_tensor("Wk", [dm, FW], bf16, kind="ExternalInput")
    Wv = nc.dram_tensor("Wv", [dm, FW], bf16, kind="ExternalInput")
    Wo = nc.dram_tensor("Wo", [FW, dm], bf16, kind="ExternalInput")
    CONST = nc.dram_tensor("CONST", [128, 257], bf16, kind="ExternalInput")
    PAD = nc.dram_tensor("PAD", [128, NB], f32, kind="ExternalInput")
    if bias_mode:
        BIAS = nc.dram_tensor("BIAS", [1, 3 * FW], bf16, kind="ExternalInput")
    Y = nc.dram_tensor("Y", [T, dm], bf16, kind="ExternalOutput")
    if DEBUG_DUMP:
        QTd = nc.dram_tensor("QTd", [128, NP * T], bf16, kind="ExternalOutput")
        KTd = nc.dram_tensor("KTd", [128, NP * T], bf16, kind="ExternalOutput")
        Vd = nc.dram_tensor("Vd", [128, NB * FW], bf16, kind="ExternalOutput")
        ATNd = nc.dram_tensor("ATNd", [128, NP * T], bf16,
                              kind="ExternalOutput")

    with TileContext(nc) as tc:
        _frees = []

        def _res(shape, dtype, name):
            t, fr = tc.tile(shape, dtype, name=name)
            _frees.append(fr)
            return t

        XQ = _res([128, C * T], bf16, "XQ")      # [p, c*T + t]
        XK = _res([128, C * T], bf16, "XK")
        XV = _res([128, C * T], bf16, "XV")
        WQS = _res([128, C * FW], bf16, "WQS")   # [p, c*FW + f]
        WKS = _res([128, C * FW], bf16, "WKS")
        WVS = _res([128, C * FW], bf16, "WVS")
        WOS = _res([128, NP * dm], bf16, "WOS")  # [p, f*dm + m]
        QT = _res([128, NP * T], bf16, "QT")     # [feat, p*T + t]
        KT = _res([128, NP * T], bf16, "KT")
        V = _res([128, NB * FW], bf16, "V")      # [k, kb*FW + f]
        ATN = _res([128, NP * T], bf16, "ATN")   # attnT [feat, p*T + t]
        CST = _res([128, 257], bf16, "CST")      # TRI | IDN | ones
        PADS = _res([128, NB], f32, "PADS")
        if bias_mode:
            BQS = _res([1, FW], bf16, "BQS")
            BKS = _res([1, FW], bf16, "BKS")
            BVS = _res([1, FW], bf16, "BVS")
            ONR = _res([1, 512], bf16, "ONR")
            nc.gpsimd.memset(ONR[:], 1.0)

        TRI = CST[:, 0:128]
        IDN = CST[:, 128:256]
        ONE = CST[:, 256:257]

        # ---- input DMAs (ordered by first use: weights, then x by quarter;
        # DMA_ENGINES is a serial resource in the cost model) ----
        def load_x_quarter(dst, src, q):
            """Load a 512-token quarter of x^T, split into two c-half DMAs
            so the first projection matmuls can start sooner."""
            dstr = dst.rearrange("p (c t) -> p c t", c=C)
            srcr = src.ap().rearrange("(c p) t -> p c t", p=128)
            for ch in range(2):
                c0, c1 = ch * (C // 2), (ch + 1) * (C // 2)
                nc.sync.dma_start(
                    dstr[:, c0:c1, q * 512:(q + 1) * 512],
                    srcr[:, c0:c1, q * 512:(q + 1) * 512])

        def load_w(dst, src):
            dstr = dst.rearrange("p (c f) -> p c f", c=C)
            srcr = src.ap().rearrange("(c p) f -> p c f", p=128)
            for ch in range(2):
                c0, c1 = ch * (C // 2), (ch + 1) * (C // 2)
                nc.sync.dma_start(dstr[:, c0:c1, :], srcr[:, c0:c1, :])

        def load_wx_half(wd, ws, xd, xs, q, ch):
            c0, c1 = ch * (C // 2), (ch + 1) * (C // 2)
            wdr = wd.rearrange("p (c f) -> p c f", c=C)
            wsr = ws.ap().rearrange("(c p) f -> p c f", p=128)
            nc.sync.dma_start(wdr[:, c0:c1, :], wsr[:, c0:c1, :])
            xdr = xd.rearrange("p (c t) -> p c t", c=C)
            xsr = xs.ap().rearrange("(c p) t -> p c t", p=128)
            nc.sync.dma_start(xdr[:, c0:c1, q * 512:(q + 1) * 512],
                              xsr[:, c0:c1, q * 512:(q + 1) * 512])

        for ch in range(2):
            load_wx_half(WQS, Wq, XQ, xqT, 0, ch)
        for ch in range(2):
            load_wx_half(WKS, Wk, XK, xkT, 0, ch)
        nc.sync.dma_start(CST[:], CONST.ap())
        nc.sync.dma_start(PADS[:], PAD.ap())
        if bias_mode:
            nc.sync.dma_start(BQS[:], BIAS.ap()[:, 0:FW])
            nc.sync.dma_start(BKS[:], BIAS.ap()[:, FW:2 * FW])
            nc.sync.dma_start(BVS[:], BIAS.ap()[:, 2 * FW:3 * FW])
        for ch in range(2):
            load_wx_half(WVS, Wv, XV, xvT, 0, ch)
        load_x_quarter(XQ, xqT, 1)
        load_x_quarter(XK, xkT, 1)
        load_x_quarter(XV, xvT, 1)
        nc.sync.dma_start(
            WOS.rearrange("p (f m) -> p f m", f=NP),
            Wo.ap().rearrange("(f p) m -> p f m", p=128))
        for q in range(2, 4):
            load_x_quarter(XQ, xqT, q)
            load_x_quarter(XK, xkT, q)
            load_x_quarter(XV, xvT, q)

        with (
            tc.tile_pool(name="stp", space="PSUM", bufs=2) as stp,
            tc.tile_pool(name="avp", space="PSUM", bufs=1) as avp,
            tc.tile_pool(name="trp", space="PSUM", bufs=1) as trp,
            tc.tile_pool(name="pjp", space="PSUM", bufs=1) as pjp,
            tc.tile_pool(name="sb", bufs=1) as sb,
        ):
            # ---------------- filler machinery ----------------
            # Fillers are small PE work units (proj / O-proj chunks) emitted
            # between attention steps so the in-order PE never drains while
            # ACT runs exp. Each has a deadline slot by which it MUST be
            # emitted (its consumer is about to be emitted); otherwise they
            # are paced by estimated PE cycles across the half's kb slots.
            filler = []          # list of (deadline_slot, cycles, fn)
            pace = {"slot": 0, "debt": 0.0}

            def pace_reset(_slots_total=None):
                pace["slot"] = 0
                pace["debt"] = 0.0

            def pace_tick(need_ns=0.0):
                """Advance one kb slot; emit due fillers plus enough
                budget-paced fillers to cover `need_ns` of PE idle time
                (the ACT-vs-PE deficit of the surrounding iteration)."""
                s = pace["slot"] = pace["slot"] + 1
                pace["debt"] = pace.get("debt", 0.0) + need_ns
                while filler and (filler[0][0] <= s or pace["debt"] > 0):
                    _, cyc, fn = filler.pop(0)
                    fn()
                    pace["debt"] -= cyc * 0.4167

            def flush_filler():
                while filler:
                    filler.pop(0)[2]()

            # ---------------- projections ----------------
            def proj_qk(W, B, dst, p, tcnk):
                """Project pair p, token chunk tcnk (256 toks) of Q or K."""
                ps = pjp.tile([128, 256], f32, name="pjps", tag="pj", bufs=1)
                x = XQ if W is WQS else XK
                for c in range(C):
                    nc.tensor.matmul(
                        ps[:], W[:, c * FW + p * 128:c * FW + (p + 1) * 128],
                        x[:, c * T + tcnk * 256:c * T + (tcnk + 1) * 256],
                        start=(c == 0), stop=(c == C - 1 and not bias_mode))
                if bias_mode:
                    nc.tensor.matmul(
                        ps[:], B[0:1, p * 128:(p + 1) * 128],
                        ONR[0:1, 0:256], start=False, stop=True)
                nc.vector.tensor_copy(
                    dst[:, p * T + tcnk * 256:p * T + (tcnk + 1) * 256], ps[:])

            def proj_v(kb, p):
                """Project V for key block kb, pair p (128 feat cols)."""
                ps = pjp.tile([128, 128], f32, name="pjps2", tag="pj", bufs=1)
                for c in range(C):
                    nc.tensor.matmul(
                        ps[:, 0:128],
                        XV[:, c * T + kb * 128:c * T + (kb + 1) * 128],
                        WVS[:, c * FW + p * 128:c * FW + (p + 1) * 128],
                        start=(c == 0), stop=(c == C - 1 and not bias_mode))
                if bias_mode:
                    nc.tensor.matmul(
                        ps[:, 0:128], ONR[0:1, 0:128],
                        BVS[0:1, p * 128:(p + 1) * 128],
                        start=False, stop=True)
                nc.vector.tensor_copy(
                    V[:, kb * FW + p * 128:kb * FW + (p + 1) * 128],
                    ps[:, 0:128])

            def oproj_ch(qb, ch):
                """Output projection chunk (qb, 512 dm cols) via pjp pool."""
                ps = pjp.tile([128, 512], f32, name="ojps", tag="pj", bufs=1)
                for fc in range(NP):
                    nc.tensor.matmul(
                        ps[:],
                        ATN[:, fc * T + qb * 128:fc * T + (qb + 1) * 128],
                        WOS[:, fc * dm + ch * 512:fc * dm + (ch + 1) * 512],
                        start=(fc == 0), stop=(fc == NP - 1))
                ysb = sb.tile([128, 512], bf16, name="ysb", tag="ysb", bufs=2)
                nc.vector.tensor_copy(ysb[:], ps[:])
                nc.sync.dma_start(
                    Y[qb * 128:(qb + 1) * 128, ch * 512:(ch + 1) * 512],
                    ysb[:])

            def oproj_half(qb, ch):
                """Half-row output projection chunk via the proj bank
                (usable while attention still owns the scores pool)."""
                ps = pjp.tile([128, 512], f32, name="ojh", tag="pj", bufs=1)
                for fc in range(NP):
                    nc.tensor.matmul(
                        ps[:],
                        ATN[:, fc * T + qb * 128:fc * T + (qb + 1) * 128],
                        WOS[:, fc * dm + ch * 512:fc * dm + (ch + 1) * 512],
                        start=(fc == 0), stop=(fc == NP - 1))
                ysb = sb.tile([128, 512], bf16, name="ysbh", tag="ysb",
                              bufs=2)
                nc.vector.tensor_copy(ysb[:], ps[:])
                nc.sync.dma_start(
                    Y[qb * 128:(qb + 1) * 128, ch * 512:(ch + 1) * 512],
                    ysb[:])

            def oproj_tail(qb):
                """Output projection for qb using the freed scores pool."""
                ps = stp.tile([128, 1024], f32, name="ojt", tag="st", bufs=2)
                for ch in range(2):
                    for fc in range(NP):
                        nc.tensor.matmul(
                            ps[:, ch * 512:(ch + 1) * 512],
                            ATN[:, fc * T + qb * 128:fc * T + (qb + 1) * 128],
                            WOS[:, fc * dm + ch * 512:fc * dm + (ch + 1) * 512],
                            start=(fc == 0), stop=(fc == NP - 1))
                ysb = sb.tile([128, dm], bf16, name="ysbt", tag="ysbt",
                              bufs=3)
                nc.vector.tensor_copy(ysb[:], ps[:])
                nc.sync.dma_start(Y[qb * 128:(qb + 1) * 128, :], ysb[:])

            # ---------------- attention ----------------
            def attention(h, qhalf, pref=(), next_h=None):
                """One head's attention sweep. `pref` carries score blocks
                prefetched by the previous head; before finishing, this head
                prefetches the next head's first two score blocks so ACT
                never idles across the head boundary. Returns the prefetch
                list for the next head."""
                p, r = h // 2, (h % 2) * 64
                kmax = HB * (qhalf + 1) - 1  # kb range 0..kmax
                av = avp.tile([128, 1024], f32, name="av", tag="av", bufs=1)
                last = False  # (per-qb tail evac experiment: regressed)

                # av layout: data qb-local off at [off*64, +64] (bank 0),
                # z at col 512+off (bank 1)
                def scores(kb, hh=h):
                    """Score matmuls + exp + diagonal mask for key block kb
                    of head hh; returns (kb, qlo, wt)."""
                    pp, rr = hh // 2, (hh % 2) * 64
                    qlo = max(kb, HB * qhalf)
                    ncols = (HB * (qhalf + 1) - qlo) * 128
                    st = stp.tile([128, 1024], f32, name="st", tag="st",
                                  bufs=2)
                    for a in range(0, ncols, 512):
                        b = min(a + 512, ncols)
                        nc.tensor.matmul(
                            st[:, a:b],
                            KT[rr:rr + 64,
                               pp * T + kb * 128:pp * T + (kb + 1) * 128],
                            QT[rr:rr + 64,
                               pp * T + qlo * 128 + a:pp * T + qlo * 128 + b],
                            start=True, stop=True)
                    wt = sb.tile([128, 1024], bf16, name="wt", tag="wt",
                                 bufs=3)
                    nc.scalar.activation(
                        wt[:, 0:ncols], st[:, 0:ncols],
                        mybir.ActivationFunctionType.Exp,
                        bias=PADS[:, kb:kb + 1], scale=0.125)
                    if qlo == kb:
                        nc.vector.tensor_mul(
                            wt[:, 0:128], wt[:, 0:128], TRI)
                    return kb, qlo, wt

                def av_block(pend):
                    """A.V accumulation consuming a scores() result. The
                    diagonal block goes last (it waits on the mask), with a
                    pace site mid-way.

                    PSUM start=True clears the has_written bits of the WHOLE
                    bank, so only the round's FIRST matmul into each av bank
                    carries start=True. Later regions' first writes use
                    start=False: their bits are clear, so the hardware
                    overwrites (per-element semantics) - exactly a fresh
                    accumulation start."""
                    kb, qlo, wt = pend
                    qbs = list(range(qlo, HB * (qhalf + 1)))
                    if qlo == kb:
                        qbs = qbs[1:] + qbs[:1]
                    for i, qb in enumerate(qbs):
                        off = qb - HB * qhalf
                        col = (qb - qlo) * 128
                        wtb = wt[:, col:col + 128]
                        nc.tensor.matmul(
                            av[:, off * 64:(off + 1) * 64], wtb,
                            V[:, kb * FW + h * 64:kb * FW + (h + 1) * 64],
                            start=(kb == 0 and i == 0), stop=(kb == qb),
                            skip_group_check=True)
                        nc.tensor.matmul(
                            av[:, 512 + off:513 + off], wtb, ONE,
                            start=(kb == 0 and i == 0), stop=(kb == qb),
                            skip_group_check=True)
                        if i == len(qbs) - 2:
                            pace_tick(0.0)  # deadline-only site

                pt = trp.tile([128, 1024], bf16, name="pt", tag="pt", bufs=1)
                pending_tr = None  # (off, nt) normalized, awaiting transpose

                def emit_tr():
                    nonlocal pending_tr
                    if pending_tr is not None:
                        off, nt = pending_tr
                        nc.tensor.transpose(
                            pt[0:64, off * 128:(off + 1) * 128], nt[:], IDN)
                        if last:
                            qb = HB * qhalf + off
                            nc.vector.tensor_copy(
                                ATN[r:r + 64, p * T + qb * 128:
                                    p * T + (qb + 1) * 128],
                                pt[0:64, off * 128:(off + 1) * 128])
                            if qb < NB - 1:
                                for ch in range(2):
                                    filler.append(
                                        (10 ** 9, OP_CYC2,
                                         lambda qb=qb, ch=ch:
                                         oproj_half(qb, ch)))
                        pending_tr = None

                # software pipeline: emit scores(kb+1) before av(kb) so the
                # in-order PE computes the next block while ACT runs exp;
                # each finished query block (qb == kb) is normalized on DVE
                # immediately and PE-transposed one iteration later.
                queue = list(pref)
                out_pref = []
                pend = queue.pop(0) if queue else scores(0)
                for kb in range(kmax + 1):
                    # ACT-vs-PE deficit of this iteration, covered by fillers
                    qlo_ = max(kb, HB * qhalf)
                    nc_ = (HB * (qhalf + 1) - qlo_) * 128
                    nqb = HB * (qhalf + 1) - qlo_
                    act_ns = nc_ * 0.8333 + 0
                    pe_ns = nc_ * 0.4167 + nqb * 58 + 55
                    pace_tick(max(0.0, act_ns - pe_ns))
                    if kb < kmax:
                        nxt = queue.pop(0) if queue else scores(kb + 1)
                    else:
                        nxt = None
                    emit_tr()
                    av_block(pend)
                    if kb >= HB * qhalf:
                        off = kb - HB * qhalf
                        rz = sb.tile([128, 1], f32, name="rz", tag="rz",
                                     bufs=4)
                        nc.vector.reciprocal(
                            rz[:], av[:, 512 + off:513 + off])
                        nt = sb.tile([128, 64], bf16, name="nt", tag="nt",
                                     bufs=4)
                        nc.vector.tensor_scalar_mul(
                            nt[:], av[:, off * 64:(off + 1) * 64], rz[:])
                        pending_tr = (off, nt)
                    pend = nxt
                if next_h is not None:
                    out_pref.append(scores(0, hh=next_h))
                    out_pref.append(scores(1, hh=next_h))
                pace_tick(150.0)
                emit_tr()
                if not last:
                    nc.vector.tensor_copy(
                        ATN[r:r + 64,
                            p * T + qhalf * 1024:p * T + (qhalf + 1) * 1024],
                        pt[0:64, :])
                return out_pref

            # ---------------- emission schedule ----------------
            BQ_ = BQS if bias_mode else None
            BK_ = BKS if bias_mode else None

            # preamble, ordered to match DMA arrival; later K/V pieces
            # become early-deadline fillers inside attention(0, 0)
            for t in (0, 1):
                proj_qk(WQS, BQ_, QT, 0, t)
            for t in (0, 1):
                proj_qk(WKS, BK_, KT, 0, t)
            for kb in range(4):
                proj_v(kb, 0)
            for t in (2, 3):
                proj_qk(WQS, BQ_, QT, 0, t)

            # half0 sweep fillers: remaining pairs' half0 proj (deadline:
            # before head 2p touches pair p), then pair 0's half1 pieces.
            QK_CYC = 256 * C          # one proj_qk chunk
            OP_CYC2 = 512 * NP        # one oproj_tail_ch chunk
            V_CYC = 128 * C           # one proj_v chunk
            OP_CYC = 512 * NP         # one oproj chunk

            # pair 0: K chunks 2-3 (kb 4-7) and V kb 4-7, due mid-head-0
            for t, dl in ((2, 2), (3, 4)):
                filler.append((dl, QK_CYC,
                               lambda t=t: proj_qk(WKS, BK_, KT, 0, t)))
            for kb in range(4, HB):
                filler.append((kb, V_CYC, lambda kb=kb: proj_v(kb, 0)))
            for pp in range(1, NP):
                dl = (2 * pp * (HB + 1)) - 4
                for t in range(4):
                    filler.append((dl, QK_CYC, lambda pp=pp, t=t: proj_qk(
                        WQS, BQ_, QT, pp, t)))
                    filler.append((dl, QK_CYC, lambda pp=pp, t=t: proj_qk(
                        WKS, BK_, KT, pp, t)))
                for kb in range(HB):
                    filler.append((dl, V_CYC,
                                   lambda pp=pp, kb=kb: proj_v(kb, pp)))
            for t in (4, 5, 6, 7):
                filler.append((10 ** 9, QK_CYC,
                               lambda t=t: proj_qk(WQS, BQ_, QT, 0, t)))
                filler.append((10 ** 9, QK_CYC,
                               lambda t=t: proj_qk(WKS, BK_, KT, 0, t)))
            for kb in range(HB, NB):
                filler.append((10 ** 9, V_CYC, lambda kb=kb: proj_v(kb, 0)))

            pace_reset()
            pref = ()
            for h in range(hg):
                pref = attention(h, 0, pref,
                                 h + 1 if h + 1 < hg else None)
            flush_filler()

            # half1 sweep fillers: remaining half1 proj + O-proj(half0)
            for pp in range(1, NP):
                dl = (2 * pp * (NB + 1)) - 6
                for t in (4, 5, 6, 7):
                    filler.append((dl, QK_CYC, lambda pp=pp, t=t: proj_qk(
                        WQS, BQ_, QT, pp, t)))
                    filler.append((dl, QK_CYC, lambda pp=pp, t=t: proj_qk(
                        WKS, BK_, KT, pp, t)))
                for kb in range(HB, NB):
                    filler.append((dl, V_CYC,
                                   lambda pp=pp, kb=kb: proj_v(kb, pp)))
            for qb in range(HB):
                for ch in range(2):
                    filler.append((10 ** 9, OP_CYC,
                                   lambda qb=qb, ch=ch: oproj_ch(qb, ch)))

            pace_reset()
            pref = ()
            for h in range(hg):
                pref = attention(h, 1, pref,
                                 h + 1 if h + 1 < hg else None)
            flush_filler()

            for qb in range(HB, NB):
                oproj_tail(qb)

            if DEBUG_DUMP:
                nc.sync.dma_start(QTd.ap(), QT[:])
                nc.sync.dma_start(KTd.ap(), KT[:])
                nc.sync.dma_start(Vd.ap(), V[:])
                nc.sync.dma_start(ATNd.ap(), ATN[:])

        for fr in reversed(_frees):
            fr()

    nc.compile()
    return nc


def _get_nc(T, dm, hg, bias_mode):
    key = (T, dm, hg, bias_mode)
    if key not in _NC_CACHE:
        _NC_CACHE[key] = build_kernel(*key)
    return _NC_CACHE[key]


def _bf16(a):
    return np.ascontiguousarray(a.astype(ml_dtypes.bfloat16))


def kernel(**inputs):
    xq = np.asarray(inputs["xq"], np.float32)
    xk = np.asarray(inputs["xk"], np.float32)
    xv = np.asarray(inputs["xv"], np.float32)
    pm = np.asarray(inputs["padding_mask"])
    Wq, bq = np.asarray(inputs["Wq"], np.float32), np.asarray(inputs["bq"], np.float32)
    Wk, bk = np.asarray(inputs["Wk"], np.float32), np.asarray(inputs["bk"], np.float32)
    Wv, bv = np.asarray(inputs["Wv"], np.float32), np.asarray(inputs["bv"], np.float32)
    Wo, bo = np.asarray(inputs["Wo"], np.float32), np.asarray(inputs["bo"], np.float32)

    N, T, dm = xq.shape
    H = Wq.shape[1] // 64
    hg = H // 2
    FW = hg * 64
    bias_mode = any(float(np.abs(b).max()) > 0 for b in (bq, bk, bv))
    n_cores = 2 * N
    assert n_cores == 8

    nc = _get_nc(T, dm, hg, bias_mode)

    # shared per-batch transposed activations
    xT = []
    for n in range(N):
        xT.append((
            _bf16(np.ascontiguousarray(xq[n].T)),
            _bf16(np.ascontiguousarray(xk[n].T)),
            _bf16(np.ascontiguousarray(xv[n].T)),
        ))
    ar = np.arange(128)
    const = np.zeros((128, 257), np.float32)
    const[:, 0:128] = (ar[:, None] <= ar[None, :]).astype(np.float32)
    const[:, 128:256] = np.eye(128, dtype=np.float32)
    const[:, 256] = 1.0
    const = const.astype(ml_dtypes.bfloat16)

    NB = T // 128
    in_maps = []
    for c in range(n_cores):
        n, g = c // 2, c % 2
        pad = np.where(np.asarray(pm[n]) == 0, -1e9, 0.0).astype(
            np.float32).reshape(NB, 128).T.copy()
        ins = {
            "xqT": xT[n][0], "xkT": xT[n][1], "xvT": xT[n][2],
            "Wq": _bf16(Wq[:, g * FW:(g + 1) * FW]),
            "Wk": _bf16(Wk[:, g * FW:(g + 1) * FW]),
            "Wv": _bf16(Wv[:, g * FW:(g + 1) * FW]),
            "Wo": _bf16(Wo[g * FW:(g + 1) * FW, :]),
            "CONST": const,
            "PAD": np.ascontiguousarray(pad),
        }
        if bias_mode:
            ins["BIAS"] = _bf16(np.concatenate(
                [bq[g * FW:(g + 1) * FW], bk[g * FW:(g + 1) * FW],
                 bv[g * FW:(g + 1) * FW]])[None, :])
        in_maps.append(ins)

    res = run_bass_kernel_spmd(nc, in_maps, list(range(n_cores)))

    out = np.empty((N, T, dm), np.float32)
    for n in range(N):
        y0 = np.asarray(res.results[2 * n]["Y"], dtype=np.float32)
        y1 = np.asarray(res.results[2 * n + 1]["Y"], dtype=np.float32)
        out[n] = y0 + y1 + bo[None, :]
    return out


# revision 73
# speedup vs baseline: 1.0055x; 1.0055x over previous
"""Multi-head causal attention (N=4, T=2048, DM=1024, H=16, D=64) on 8 trn2 cores.

Sharding: core = (batch, head-half). Each core computes 8 of the 16 heads for
one batch over ALL 2048 queries: Q/K/V projections restricted to its 512
features, full causal attention for its heads, and a PARTIAL output
projection attn_g @ Wo[g-rows]. The two cores of a batch produce partial Y's
that the host sums (O-projection is linear over head groups) - no collectives
and no duplicated projection work.

On-chip: activations arrive host-transposed x^T [dm, tok] bf16 so Q/K project
directly to [feat, tok] (weights stationary); V projects naturally [tok, feat].
Scores are St[k, q] = Kt_h^T . Qt_h per key block, one fused exp (ACT) per
(head, q-half, kb) with the key-padding bias per partition; the causal
diagonal is a binary bf16 mask multiplied in on DVE. The A.V matmul is
FLIPPED: stationary = wt block [128k,128q], moving = V slice [128k,64] plus
a ones column producing Z - so each accumulation step charges only 65 output
columns. The 8 query-block accumulators share two PSUM banks; since
start=True clears the has_written bits of a WHOLE bank, only the round's
first matmul per bank starts (other regions' first writes rely on the
per-element overwrite-where-unset semantics). Per finished query block, a
reciprocal + tensor_scalar normalize (DVE) feeds a PE transpose back to
[feat, tok] for the output projection. Projections and O-proj chunks are
emitted as deadline/deficit-paced FILLER between attention steps (software-
pipelined scores->exp->A.V) so the in-order PE never drains while ACT runs
exp.
"""

import numpy as np
import ml_dtypes

import concourse.mybir as mybir
from concourse import bacc
from concourse.tile import TileContext
from concourse.bass_utils import run_bass_kernel_spmd

bf16 = mybir.dt.bfloat16
f32 = mybir.dt.float32

_NC_CACHE = {}
DEBUG_DUMP = False   # add QT/KT/V/ATN dram dumps for debugging

# T tokens, DM model dim, HG heads per core (group), D head dim
BUILD_KEY = (2048, 1024, 8, False)


def build_kernel(T, dm, hg, bias_mode):
    d = 64
    NB = T // 128          # token blocks (16)
    C = dm // 128          # contraction chunks (8)
    NP = hg // 2           # head pairs per core (4)
    FW = hg * d            # feature width per core (512)
    HB = NB // 2           # blocks per half (8)

    nc = bacc.Bacc("TRN2", target_bir_lowering=False, debug=False)

    xqT = nc.dram_tensor("xqT", [dm, T], bf16, kind="ExternalInput")
    xkT = nc.dram_tensor("xkT", [dm, T], bf16, kind="ExternalInput")
    xvT = nc.dram_tensor("xvT", [dm, T], bf16, kind="ExternalInput")
    Wq = nc.dram_tensor("Wq", [dm, FW], bf16, kind="ExternalInput")
    Wk = nc.dram_tensor("Wk", [dm, FW], bf16, kind="ExternalInput")
    Wv = nc.dram_tensor("Wv", [dm, FW], bf16, kind="ExternalInput")
    Wo = nc.dram_tensor("Wo", [FW, dm], bf16, kind="ExternalInput")
    CONST = nc.dram_tensor("CONST", [128, 257], bf16, kind="ExternalInput")
    PAD = nc.dram_tensor("PAD", [128, NB], f32, kind="ExternalInput")
    if bias_mode:
        BIAS = nc.dram_tensor("BIAS", [1, 3 * FW], bf16, kind="ExternalInput")
    Y = nc.dram_tensor("Y", [T, dm], bf16, kind="ExternalOutput")
    if DEBUG_DUMP:
        QTd = nc.dram_tensor("QTd", [128, NP * T], bf16, kind="ExternalOutput")
        KTd = nc.dram_tensor("KTd", [128, NP * T], bf16, kind="ExternalOutput")
        Vd = nc.dram_tensor("Vd", [128, NB * FW], bf16, kind="ExternalOutput")
        ATNd = nc.dram_tensor("ATNd", [128, NP * T], bf16,
                              kind="ExternalOutput")

    with TileContext(nc) as tc:
        _frees = []

        def _res(shape, dtype, name):
            t, fr = tc.tile(shape, dtype, name=name)
            _frees.append(fr)
            return t

        XQ = _res([128, C * T], bf16, "XQ")      # [p, c*T + t]
        XK = _res([128, C * T], bf16, "XK")
        XV = _res([128, C * T], bf16, "XV")
        WQS = _res([128, C * FW], bf16, "WQS")   # [p, c*FW + f]
        WKS = _res([128, C * FW], bf16, "WKS")
        WVS = _res([128, C * FW], bf16, "WVS")
        WOS = _res([128, NP * dm], bf16, "WOS")  # [p, f*dm + m]
        QT = _res([128, NP * T], bf16, "QT")     # [feat, p*T + t]
        KT = _res([128, NP * T], bf16, "KT")
        V = _res([128, NB * FW], bf16, "V")      # [k, kb*FW + f]
        ATN = _res([128, NP * T], bf16, "ATN")   # attnT [feat, p*T + t]
        CST = _res([128, 257], bf16, "CST")      # TRI | IDN | ones
        PADS = _res([128, NB], f32, "PADS")
        if bias_mode:
            BQS = _res([1, FW], bf16, "BQS")
            BKS = _res([1, FW], bf16, "BKS")
            BVS = _res([1, FW], bf16, "BVS")
            ONR = _res([1, 512], bf16, "ONR")
            nc.gpsimd.memset(ONR[:], 1.0)

        TRI = CST[:, 0:128]
        IDN = CST[:, 128:256]
        ONE = CST[:, 256:257]

        # ---- input DMAs (ordered by first use: weights, then x by quarter;
        # DMA_ENGINES is a serial resource in the cost model) ----
        def load_x_quarter(dst, src, q):
            """Load a 512-token quarter of x^T as two 256-token sub-DMAs
            (all contraction chunks): the first sub already enables full
            projection of its token blocks, halving first-use latency."""
            dstr = dst.rearrange("p (c t) -> p c t", c=C)
            srcr = src.ap().rearrange("(c p) t -> p c t", p=128)
            for sub in range(2):
                t0 = q * 512 + sub * 256
                nc.sync.dma_start(
                    dstr[:, :, t0:t0 + 256],
                    srcr[:, :, t0:t0 + 256])

        def load_w(dst, src):
            dstr = dst.rearrange("p (c f) -> p c f", c=C)
            srcr = src.ap().rearrange("(c p) f -> p c f", p=128)
            for ch in range(2):
                c0, c1 = ch * (C // 2), (ch + 1) * (C // 2)
                nc.sync.dma_start(dstr[:, c0:c1, :], srcr[:, c0:c1, :])

        def load_w_half(wd, ws, ch):
            c0, c1 = ch * (C // 2), (ch + 1) * (C // 2)
            wdr = wd.rearrange("p (c f) -> p c f", c=C)
            wsr = ws.ap().rearrange("(c p) f -> p c f", p=128)
            nc.sync.dma_start(wdr[:, c0:c1, :], wsr[:, c0:c1, :])

        def load_wx_half(wd, ws, xd, xs, q, ch):
            load_w_half(wd, ws, ch)
            xdr = xd.rearrange("p (c t) -> p c t", c=C)
            xsr = xs.ap().rearrange("(c p) t -> p c t", p=128)
            t0 = q * 512 + ch * 256
            nc.sync.dma_start(xdr[:, :, t0:t0 + 256],
                              xsr[:, :, t0:t0 + 256])

        for ch in range(2):
            load_wx_half(WQS, Wq, XQ, xqT, 0, ch)
        for ch in range(2):
            load_wx_half(WKS, Wk, XK, xkT, 0, ch)
        nc.sync.dma_start(CST[:], CONST.ap())
        nc.sync.dma_start(PADS[:], PAD.ap())
        if bias_mode:
            nc.sync.dma_start(BQS[:], BIAS.ap()[:, 0:FW])
            nc.sync.dma_start(BKS[:], BIAS.ap()[:, FW:2 * FW])
            nc.sync.dma_start(BVS[:], BIAS.ap()[:, 2 * FW:3 * FW])
        for ch in range(2):
            load_wx_half(WVS, Wv, XV, xvT, 0, ch)
        load_x_quarter(XQ, xqT, 1)
        load_x_quarter(XK, xkT, 1)
        load_x_quarter(XV, xvT, 1)
        nc.sync.dma_start(
            WOS.rearrange("p (f m) -> p f m", f=NP),
            Wo.ap().rearrange("(f p) m -> p f m", p=128))
        for q in range(2, 4):
            load_x_quarter(XQ, xqT, q)
            load_x_quarter(XK, xkT, q)
            load_x_quarter(XV, xvT, q)

        with (
            tc.tile_pool(name="stp", space="PSUM", bufs=2) as stp,
            tc.tile_pool(name="avp", space="PSUM", bufs=1) as avp,
            tc.tile_pool(name="trp", space="PSUM", bufs=1) as trp,
            tc.tile_pool(name="pjp", space="PSUM", bufs=1) as pjp,
            tc.tile_pool(name="sb", bufs=1) as sb,
        ):
            # ---------------- filler machinery ----------------
            # Fillers are small PE work units (proj / O-proj chunks) emitted
            # between attention steps so the in-order PE never drains while
            # ACT runs exp. Each has a deadline slot by which it MUST be
            # emitted (its consumer is about to be emitted); otherwise they
            # are paced by estimated PE cycles across the half's kb slots.
            filler = []          # list of (deadline_slot, cycles, fn)
            pace = {"slot": 0, "debt": 0.0}

            def pace_reset(_slots_total=None):
                pace["slot"] = 0
                pace["debt"] = 0.0

            def pace_tick(need_ns=0.0):
                """Advance one kb slot; emit due fillers plus enough
                budget-paced fillers to cover `need_ns` of PE idle time
                (the ACT-vs-PE deficit of the surrounding iteration)."""
                s = pace["slot"] = pace["slot"] + 1
                pace["debt"] = pace.get("debt", 0.0) + need_ns
                while filler and (filler[0][0] <= s or pace["debt"] > 0):
                    _, cyc, fn = filler.pop(0)
                    fn()
                    pace["debt"] -= cyc * 0.4167

            def flush_filler():
                while filler:
                    filler.pop(0)[2]()

            # ---------------- projections ----------------
            def proj_qk(W, B, dst, p, tcnk):
                """Project pair p, token chunk tcnk (256 toks) of Q or K."""
                ps = pjp.tile([128, 256], f32, name="pjps", tag="pj", bufs=1)
                x = XQ if W is WQS else XK
                for c in range(C):
                    nc.tensor.matmul(
                        ps[:], W[:, c * FW + p * 128:c * FW + (p + 1) * 128],
                        x[:, c * T + tcnk * 256:c * T + (tcnk + 1) * 256],
                        start=(c == 0), stop=(c == C - 1 and not bias_mode))
                if bias_mode:
                    nc.tensor.matmul(
                        ps[:], B[0:1, p * 128:(p + 1) * 128],
                        ONR[0:1, 0:256], start=False, stop=True)
                nc.vector.tensor_copy(
                    dst[:, p * T + tcnk * 256:p * T + (tcnk + 1) * 256], ps[:])

            def proj_v(kb, p):
                """Project V for key block kb, pair p (128 feat cols)."""
                ps = pjp.tile([128, 128], f32, name="pjps2", tag="pj", bufs=1)
                for c in range(C):
                    nc.tensor.matmul(
                        ps[:, 0:128],
                        XV[:, c * T + kb * 128:c * T + (kb + 1) * 128],
                        WVS[:, c * FW + p * 128:c * FW + (p + 1) * 128],
                        start=(c == 0), stop=(c == C - 1 and not bias_mode))
                if bias_mode:
                    nc.tensor.matmul(
                        ps[:, 0:128], ONR[0:1, 0:128],
                        BVS[0:1, p * 128:(p + 1) * 128],
                        start=False, stop=True)
                nc.vector.tensor_copy(
                    V[:, kb * FW + p * 128:kb * FW + (p + 1) * 128],
                    ps[:, 0:128])

            def oproj_ch(qb, ch):
                """Output projection chunk (qb, 512 dm cols) via pjp pool."""
                ps = pjp.tile([128, 512], f32, name="ojps", tag="pj", bufs=1)
                for fc in range(NP):
                    nc.tensor.matmul(
                        ps[:],
                        ATN[:, fc * T + qb * 128:fc * T + (qb + 1) * 128],
                        WOS[:, fc * dm + ch * 512:fc * dm + (ch + 1) * 512],
                        start=(fc == 0), stop=(fc == NP - 1))
                ysb = sb.tile([128, 512], bf16, name="ysb", tag="ysb", bufs=2)
                nc.vector.tensor_copy(ysb[:], ps[:])
                nc.sync.dma_start(
                    Y[qb * 128:(qb + 1) * 128, ch * 512:(ch + 1) * 512],
                    ysb[:])

            def oproj_half(qb, ch):
                """Half-row output projection chunk via the proj bank
                (usable while attention still owns the scores pool)."""
                ps = pjp.tile([128, 512], f32, name="ojh", tag="pj", bufs=1)
                for fc in range(NP):
                    nc.tensor.matmul(
                        ps[:],
                        ATN[:, fc * T + qb * 128:fc * T + (qb + 1) * 128],
                        WOS[:, fc * dm + ch * 512:fc * dm + (ch + 1) * 512],
                        start=(fc == 0), stop=(fc == NP - 1))
                ysb = sb.tile([128, 512], bf16, name="ysbh", tag="ysb",
                              bufs=2)
                nc.vector.tensor_copy(ysb[:], ps[:])
                nc.sync.dma_start(
                    Y[qb * 128:(qb + 1) * 128, ch * 512:(ch + 1) * 512],
                    ysb[:])

            def oproj_tail(qb):
                """Output projection for qb using the freed scores pool."""
                ps = stp.tile([128, 1024], f32, name="ojt", tag="st", bufs=2)
                for ch in range(2):
                    for fc in range(NP):
                        nc.tensor.matmul(
                            ps[:, ch * 512:(ch + 1) * 512],
                            ATN[:, fc * T + qb * 128:fc * T + (qb + 1) * 128],
                            WOS[:, fc * dm + ch * 512:fc * dm + (ch + 1) * 512],
                            start=(fc == 0), stop=(fc == NP - 1))
                ysb = sb.tile([128, dm], bf16, name="ysbt", tag="ysbt",
                              bufs=3)
                nc.vector.tensor_copy(ysb[:], ps[:])
                nc.sync.dma_start(Y[qb * 128:(qb + 1) * 128, :], ysb[:])

            # ---------------- attention ----------------
            def attention(h, qhalf, pref=(), next_h=None):
                """One head's attention sweep. `pref` carries score blocks
                prefetched by the previous head; before finishing, this head
                prefetches the next head's first two score blocks so ACT
                never idles across the head boundary. Returns the prefetch
                list for the next head."""
                p, r = h // 2, (h % 2) * 64
                kmax = HB * (qhalf + 1) - 1  # kb range 0..kmax
                av = avp.tile([128, 1024], f32, name="av", tag="av", bufs=1)
                last = False  # (per-qb tail evac experiment: regressed)

                # av layout: data qb-local off at [off*64, +64] (bank 0),
                # z at col 512+off (bank 1)
                def scores(kb, hh=h):
                    """Score matmuls + exp + diagonal mask for key block kb
                    of head hh; returns (kb, qlo, wt)."""
                    pp, rr = hh // 2, (hh % 2) * 64
                    qlo = max(kb, HB * qhalf)
                    ncols = (HB * (qhalf + 1) - qlo) * 128
                    st = stp.tile([128, 1024], f32, name="st", tag="st",
                                  bufs=2)
                    for a in range(0, ncols, 512):
                        b = min(a + 512, ncols)
                        nc.tensor.matmul(
                            st[:, a:b],
                            KT[rr:rr + 64,
                               pp * T + kb * 128:pp * T + (kb + 1) * 128],
                            QT[rr:rr + 64,
                               pp * T + qlo * 128 + a:pp * T + qlo * 128 + b],
                            start=True, stop=True)
                    wt = sb.tile([128, 1024], bf16, name="wt", tag="wt",
                                 bufs=3)
                    nc.scalar.activation(
                        wt[:, 0:ncols], st[:, 0:ncols],
                        mybir.ActivationFunctionType.Exp,
                        bias=PADS[:, kb:kb + 1], scale=0.125)
                    if qlo == kb:
                        nc.vector.tensor_mul(
                            wt[:, 0:128], wt[:, 0:128], TRI)
                    return kb, qlo, wt

                def av_block(pend):
                    """A.V accumulation consuming a scores() result. The
                    diagonal block goes last (it waits on the mask), with a
                    pace site mid-way.

                    PSUM start=True clears the has_written bits of the WHOLE
                    bank, so only the round's FIRST matmul into each av bank
                    carries start=True. Later regions' first writes use
                    start=False: their bits are clear, so the hardware
                    overwrites (per-element semantics) - exactly a fresh
                    accumulation start."""
                    kb, qlo, wt = pend
                    qbs = list(range(qlo, HB * (qhalf + 1)))
                    if qlo == kb:
                        qbs = qbs[1:] + qbs[:1]
                    for i, qb in enumerate(qbs):
                        off = qb - HB * qhalf
                        col = (qb - qlo) * 128
                        wtb = wt[:, col:col + 128]
                        nc.tensor.matmul(
                            av[:, off * 64:(off + 1) * 64], wtb,
                            V[:, kb * FW + h * 64:kb * FW + (h + 1) * 64],
                            start=(kb == 0 and i == 0), stop=(kb == qb),
                            skip_group_check=True)
                        nc.tensor.matmul(
                            av[:, 512 + off:513 + off], wtb, ONE,
                            start=(kb == 0 and i == 0), stop=(kb == qb),
                            skip_group_check=True)
                        if i == len(qbs) - 2:
                            pace_tick(0.0)  # deadline-only site

                pt = trp.tile([128, 1024], bf16, name="pt", tag="pt", bufs=1)
                pending_tr = None  # (off, nt) normalized, awaiting transpose

                def emit_tr():
                    nonlocal pending_tr
                    if pending_tr is not None:
                        off, nt = pending_tr
                        nc.tensor.transpose(
                            pt[0:64, off * 128:(off + 1) * 128], nt[:], IDN)
                        if last:
                            qb = HB * qhalf + off
                            nc.vector.tensor_copy(
                                ATN[r:r + 64, p * T + qb * 128:
                                    p * T + (qb + 1) * 128],
                                pt[0:64, off * 128:(off + 1) * 128])
                            if qb < NB - 1:
                                for ch in range(2):
                                    filler.append(
                                        (10 ** 9, OP_CYC2,
                                         lambda qb=qb, ch=ch:
                                         oproj_half(qb, ch)))
                        pending_tr = None

                # software pipeline: emit scores(kb+1) before av(kb) so the
                # in-order PE computes the next block while ACT runs exp;
                # each finished query block (qb == kb) is normalized on DVE
                # immediately and PE-transposed one iteration later.
                queue = list(pref)
                out_pref = []
                pend = queue.pop(0) if queue else scores(0)
                for kb in range(kmax + 1):
                    # ACT-vs-PE deficit of this iteration, covered by fillers
                    qlo_ = max(kb, HB * qhalf)
                    nc_ = (HB * (qhalf + 1) - qlo_) * 128
                    nqb = HB * (qhalf + 1) - qlo_
                    act_ns = nc_ * 0.8333 + 0
                    pe_ns = nc_ * 0.4167 + nqb * 58 + 55
                    pace_tick(max(0.0, act_ns - pe_ns))
                    if kb < kmax:
                        nxt = queue.pop(0) if queue else scores(kb + 1)
                    else:
                        nxt = None
                    emit_tr()
                    av_block(pend)
                    if kb >= HB * qhalf:
                        off = kb - HB * qhalf
                        rz = sb.tile([128, 1], f32, name="rz", tag="rz",
                                     bufs=4)
                        nc.vector.reciprocal(
                            rz[:], av[:, 512 + off:513 + off])
                        nt = sb.tile([128, 64], bf16, name="nt", tag="nt",
                                     bufs=4)
                        nc.vector.tensor_scalar_mul(
                            nt[:], av[:, off * 64:(off + 1) * 64], rz[:])
                        pending_tr = (off, nt)
                    pend = nxt
                if next_h is not None:
                    out_pref.append(scores(0, hh=next_h))
                    out_pref.append(scores(1, hh=next_h))
                pace_tick(150.0)
                emit_tr()
                if not last:
                    nc.vector.tensor_copy(
                        ATN[r:r + 64,
                            p * T + qhalf * 1024:p * T + (qhalf + 1) * 1024],
                        pt[0:64, :])
                return out_pref

            # ---------------- emission schedule ----------------
            BQ_ = BQS if bias_mode else None
            BK_ = BKS if bias_mode else None

            # preamble, ordered to match DMA arrival; later K/V pieces
            # become early-deadline fillers inside attention(0, 0)
            for t in (0, 1):
                proj_qk(WQS, BQ_, QT, 0, t)
            for t in (0, 1):
                proj_qk(WKS, BK_, KT, 0, t)
            for kb in range(4):
                proj_v(kb, 0)
            for t in (2, 3):
                proj_qk(WQS, BQ_, QT, 0, t)

            # half0 sweep fillers: remaining pairs' half0 proj (deadline:
            # before head 2p touches pair p), then pair 0's half1 pieces.
            QK_CYC = 256 * C          # one proj_qk chunk
            OP_CYC2 = 512 * NP        # one oproj_tail_ch chunk
            V_CYC = 128 * C           # one proj_v chunk
            OP_CYC = 512 * NP         # one oproj chunk

            # pair 0: K chunks 2-3 (kb 4-7) and V kb 4-7, due mid-head-0
            for t, dl in ((2, 2), (3, 4)):
                filler.append((dl, QK_CYC,
                               lambda t=t: proj_qk(WKS, BK_, KT, 0, t)))
            for kb in range(4, HB):
                filler.append((kb, V_CYC, lambda kb=kb: proj_v(kb, 0)))
            for pp in range(1, NP):
                dl = (2 * pp * (HB + 1)) - 4
                for t in range(4):
                    filler.append((dl, QK_CYC, lambda pp=pp, t=t: proj_qk(
                        WQS, BQ_, QT, pp, t)))
                    filler.append((dl, QK_CYC, lambda pp=pp, t=t: proj_qk(
                        WKS, BK_, KT, pp, t)))
                for kb in range(HB):
                    filler.append((dl, V_CYC,
                                   lambda pp=pp, kb=kb: proj_v(kb, pp)))
            for t in (4, 5, 6, 7):
                filler.append((10 ** 9, QK_CYC,
                               lambda t=t: proj_qk(WQS, BQ_, QT, 0, t)))
                filler.append((10 ** 9, QK_CYC,
                               lambda t=t: proj_qk(WKS, BK_, KT, 0, t)))
            for kb in range(HB, NB):
                filler.append((10 ** 9, V_CYC, lambda kb=kb: proj_v(kb, 0)))

            pace_reset()
            pref = ()
            for h in range(hg):
                pref = attention(h, 0, pref,
                                 h + 1 if h + 1 < hg else None)
            flush_filler()

            # half1 sweep fillers: remaining half1 proj + O-proj(half0)
            for pp in range(1, NP):
                dl = (2 * pp * (NB + 1)) - 6
                for t in (4, 5, 6, 7):
                    filler.append((dl, QK_CYC, lambda pp=pp, t=t: proj_qk(
                        WQS, BQ_, QT, pp, t)))
                    filler.append((dl, QK_CYC, lambda pp=pp, t=t: proj_qk(
                        WKS, BK_, KT, pp, t)))
                for kb in range(HB, NB):
                    filler.append((dl, V_CYC,
                                   lambda pp=pp, kb=kb: proj_v(kb, pp)))
            for qb in range(HB):
                for ch in range(2):
                    filler.append((10 ** 9, OP_CYC,
                                   lambda qb=qb, ch=ch: oproj_ch(qb, ch)))

            pace_reset()
            pref = ()
            for h in range(hg):
                pref = attention(h, 1, pref,
                                 h + 1 if h + 1 < hg else None)
            flush_filler()

            for qb in range(HB, NB):
                oproj_tail(qb)

            if DEBUG_DUMP:
                nc.sync.dma_start(QTd.ap(), QT[:])
                nc.sync.dma_start(KTd.ap(), KT[:])
                nc.sync.dma_start(Vd.ap(), V[:])
                nc.sync.dma_start(ATNd.ap(), ATN[:])

        for fr in reversed(_frees):
            fr()

    nc.compile()
    return nc


def _get_nc(T, dm, hg, bias_mode):
    key = (T, dm, hg, bias_mode)
    if key not in _NC_CACHE:
        _NC_CACHE[key] = build_kernel(*key)
    return _NC_CACHE[key]


def _bf16(a):
    return np.ascontiguousarray(a.astype(ml_dtypes.bfloat16))


def kernel(**inputs):
    xq = np.asarray(inputs["xq"], np.float32)
    xk = np.asarray(inputs["xk"], np.float32)
    xv = np.asarray(inputs["xv"], np.float32)
    pm = np.asarray(inputs["padding_mask"])
    Wq, bq = np.asarray(inputs["Wq"], np.float32), np.asarray(inputs["bq"], np.float32)
    Wk, bk = np.asarray(inputs["Wk"], np.float32), np.asarray(inputs["bk"], np.float32)
    Wv, bv = np.asarray(inputs["Wv"], np.float32), np.asarray(inputs["bv"], np.float32)
    Wo, bo = np.asarray(inputs["Wo"], np.float32), np.asarray(inputs["bo"], np.float32)

    N, T, dm = xq.shape
    H = Wq.shape[1] // 64
    hg = H // 2
    FW = hg * 64
    bias_mode = any(float(np.abs(b).max()) > 0 for b in (bq, bk, bv))
    n_cores = 2 * N
    assert n_cores == 8

    nc = _get_nc(T, dm, hg, bias_mode)

    # shared per-batch transposed activations
    xT = []
    for n in range(N):
        xT.append((
            _bf16(np.ascontiguousarray(xq[n].T)),
            _bf16(np.ascontiguousarray(xk[n].T)),
            _bf16(np.ascontiguousarray(xv[n].T)),
        ))
    ar = np.arange(128)
    const = np.zeros((128, 257), np.float32)
    const[:, 0:128] = (ar[:, None] <= ar[None, :]).astype(np.float32)
    const[:, 128:256] = np.eye(128, dtype=np.float32)
    const[:, 256] = 1.0
    const = const.astype(ml_dtypes.bfloat16)

    NB = T // 128
    in_maps = []
    for c in range(n_cores):
        n, g = c // 2, c % 2
        pad = np.where(np.asarray(pm[n]) == 0, -1e9, 0.0).astype(
            np.float32).reshape(NB, 128).T.copy()
        ins = {
            "xqT": xT[n][0], "xkT": xT[n][1], "xvT": xT[n][2],
            "Wq": _bf16(Wq[:, g * FW:(g + 1) * FW]),
            "Wk": _bf16(Wk[:, g * FW:(g + 1) * FW]),
            "Wv": _bf16(Wv[:, g * FW:(g + 1) * FW]),
            "Wo": _bf16(Wo[g * FW:(g + 1) * FW, :]),
            "CONST": const,
            "PAD": np.ascontiguousarray(pad),
        }
        if bias_mode:
            ins["BIAS"] = _bf16(np.concatenate(
                [bq[g * FW:(g + 1) * FW], bk[g * FW:(g + 1) * FW],
                 bv[g * FW:(g + 1) * FW]])[None, :])
        in_maps.append(ins)

    res = run_bass_kernel_spmd(nc, in_maps, list(range(n_cores)))

    out = np.empty((N, T, dm), np.float32)
    for n in range(N):
        y0 = np.asarray(res.results[2 * n]["Y"], dtype=np.float32)
        y1 = np.asarray(res.results[2 * n + 1]["Y"], dtype=np.float32)
        out[n] = y0 + y1 + bo[None, :]
    return out
